# revision 1
# baseline (speedup 1.0000x reference)
"""Trainium2 Bass kernel for scatter_memory problem nn_Memory_value_57475252355404.

out[b, dispatch[b,e,c], :] += weight[indices[b,e,c], :] * score[b,e,c]

Strategy (8 cores, SPMD single program, ONE launch):
  - Shard OUTPUT rows n across cores: core k owns out[:, k*512:(k+1)*512, :].
  - Host: route each token to its owning core (dispatch // 512); within a
    core, tokens go to one of 8 SWDGE gather buckets by index window
    (dma_gather idxs are int16, addressing a 32768-row window per call).
  - Table is converted to bf16 on host: halves gather bytes, enables
    full-rate bf16 matmuls (fp32 matmuls are 4x slower on the PE).
  - Scatter-add via per-block one-hot bf16 matmuls: block g = 128 tokens;
    each distinct dest row in the block gets a rank slot in [0,128); a fused
    DVE op builds onehot[t, r] = (iota[r] == destrel[t]) * score[t]
    (all precomputed up front); the PE computes psum[d, r] = sum_t
    tok[t, d] * onehot[t, r], 4 groups per PSUM bank; the activation engine
    copies each bank to a bf16 rank-space buffer, DMA'd out per bank.
  - Host: rank slots -> physical rows (np.add.at in f32), concat 8 slices.
"""

import sys

sys.path.insert(0, "/opt/trn_rl_repo")

import numpy as np
import ml_dtypes

BF16 = ml_dtypes.bfloat16

B, E, C = 4, 16, 512
EC = E * C
V, D = 262144, 128
N = 4096
NCORES = 8
NSLICE = N // NCORES  # 512
NBUCKETS = 8
BUCKET = V // NBUCKETS  # 32768

_cache = {}
LAST_RESULTS = None  # BassKernelResults of the most recent run (for test.py)


def _build(cap, order):
    """Build+compile the SPMD program. `order` = emission sequence of
    (bucket, queue) for the 8 gather calls, chosen to balance per-queue
    desc-gen load (gen time follows token count; the two calls sharing a
    queue run back-to-back on its ucode worker)."""
    from concourse import bacc, tile, mybir, library_config

    f32 = mybir.dt.float32
    bf16 = mybir.dt.bfloat16
    i16 = mybir.dt.int16
    i32 = mybir.dt.int32

    G_c = cap // 128  # groups per SWDGE bucket
    G = NBUCKETS * G_c  # total groups
    TOT = G * 128

    nc = bacc.Bacc(
        "TRN2",
        target_bir_lowering=False,
        debug=False,
        num_devices=NCORES,
        num_swdge_queues=4,
    )
    w = nc.dram_tensor("weight", [V, D], bf16, kind="ExternalInput")
    gi = nc.dram_tensor("gidx", [128, NBUCKETS * cap // 16], i16, kind="ExternalInput")
    # meta = score_s [128, G] | destrel [128, G] (f32: tensor_scalar scalar
    # operands must be f32 for is_equal)
    meta = nc.dram_tensor("meta", [128, 2 * G], f32, kind="ExternalInput")
    iota = nc.dram_tensor("iota128", [128, 128], bf16, kind="ExternalInput")
    cnt = nc.dram_tensor("cnt", [1, NBUCKETS], i32, kind="ExternalInput")
    out = nc.dram_tensor("out", [128, TOT], bf16, kind="ExternalOutput")

    with tile.TileContext(nc) as tc:
        with tc.tile_pool(name="p", bufs=1) as pool, \
             tc.tile_pool(name="oh", bufs=G) as ohp, \
             tc.tile_pool(name="ps", bufs=8, space="PSUM") as psp:
            # start the gpsimd ucode library load (~9us) as early as possible;
            # the dma_gather desc-gen can't run until it completes
            nc.gpsimd.load_library(library_config.mlp)
            cnt_t = pool.tile([1, NBUCKETS], i32)
            nc.sync.dma_start(cnt_t[:], cnt.ap())
            gi_t = pool.tile([128, NBUCKETS * cap // 16], i16)
            nc.sync.dma_start(gi_t[:], gi.ap())
            meta_t = pool.tile([128, 2 * G], f32)
            nc.sync.dma_start(meta_t[:], meta.ap())
            io_t = pool.tile([128, 128], bf16)
            nc.sync.dma_start(io_t[:], iota.ap())
            sc_t = meta_t[:, 0:G]
            dr_t = meta_t[:, G : 2 * G]
            iob = io_t[:]

            tok = pool.tile([128, G, D], bf16)

            _, cnt_vals = nc.values_load_multi_w_load_instructions(
                cnt_t[0:1, 0:NBUCKETS],
                engines=[mybir.EngineType.Pool],
                min_val=1,
                max_val=cap,
                skip_runtime_bounds_check=True,
            )

            # pad positions (beyond each bucket's valid count) are never
            # written by the gather; zero them so 0-weighted onehot rows
            # can't pull NaN/Inf garbage into the psum accumulation
            nc.vector.memset(tok[:], 0)
            wap = w.ap()
            for k, q in order:
                nc.gpsimd.dma_gather(
                    tok[:, k * G_c : (k + 1) * G_c, :],
                    wap[k * BUCKET : (k + 1) * BUCKET, :],
                    gi_t[:, k * (cap // 16) : (k + 1) * (cap // 16)],
                    cap,
                    cnt_vals[k],
                    D,
                    queue_num=q,
                )

            # precompute ALL onehots up front: they depend only on meta/iota,
            # so the DVE builds them during the gather wait and the PE never
            # stalls on a WAR-chained onehot
            ohs = []
            for g in range(G):
                oh = ohp.tile([128, 128], bf16, tag="oh")
                # onehot[t, r] = (iota[r] == destrel[t]) * score[t]
                nc.vector.tensor_scalar(
                    out=oh[:],
                    in0=iob,
                    scalar1=dr_t[:, g : g + 1],
                    scalar2=sc_t[:, g : g + 1],
                    op0=mybir.AluOpType.is_equal,
                    op1=mybir.AluOpType.mult,
                )
                ohs.append(oh)

            osb = pool.tile([128, TOT], bf16)
            # 4 groups share one PSUM bank; one batched ACT copy per bank
            # keeps the copy engine off the critical path; out-DMA per bank
            oap = out.ap()
            for b in range((G + 3) // 4):
                glo = b * 4
                ghi = min(glo + 4, G)
                span = ghi - glo
                ps = psp.tile([128, 512], f32, tag="ps")
                for j in range(span):
                    g = glo + j
                    nc.tensor.matmul(
                        ps[:, j * 128 : (j + 1) * 128],
                        tok[:, g, :],
                        ohs[g][:],
                        start=True,
                        stop=True,
                    )
                nc.scalar.activation(
                    osb[:, glo * 128 : ghi * 128],
                    ps[:, 0 : span * 128],
                    mybir.ActivationFunctionType.Copy,
                )
                nc.sync.dma_start(
                    oap[:, glo * 128 : ghi * 128], osb[:, glo * 128 : ghi * 128]
                )

    nc.compile()
    return nc


def _wrap16(a):
    """[M] -> [16, M/16] wrap (token j at [j%16, j//16]) replicated to 128 parts."""
    m = a.shape[0]
    w = a.reshape(m // 16, 16).T  # [16, M/16]
    return np.tile(w, (8, 1)).copy()  # [128, M/16]


def _preprocess(score, indices, dispatch, weight):
    sc = np.ascontiguousarray(np.asarray(score, dtype=np.float32)).reshape(B, EC)
    ix = np.asarray(indices).astype(np.int64, copy=False).reshape(B, EC)
    dp = np.asarray(dispatch).astype(np.int64, copy=False).reshape(B, EC)

    flat_core = (dp // NSLICE).ravel()
    flat_bucket = (ix // BUCKET).ravel()
    flat_b = np.repeat(np.arange(B, dtype=np.int64), EC)
    flat_ix = ix.ravel()
    # dest row within the core's [B*NSLICE] local output space
    flat_dest = (flat_b * NSLICE + (dp % NSLICE).ravel()).astype(np.int64)
    flat_sc = sc.ravel()

    counts = np.zeros((NCORES, NBUCKETS), np.int64)
    np.add.at(counts, (flat_core, flat_bucket), 1)
    cap = int(np.ceil(max(int(counts.max()), 128) / 128.0) * 128)
    TOT = NBUCKETS * cap
    G = TOT // 128

    # stable sort by (core, bucket, dest): dest-sorted within each bucket
    # maximizes rank compression within blocks (fewer host-side adds) and
    # keeps each (core,bucket) group contiguous for the gather call.
    key = (flat_core * NBUCKETS + flat_bucket) * (B * NSLICE) + flat_dest
    order = np.argsort(key, kind="stable")
    s_core = flat_core[order]
    s_bucket = flat_bucket[order]
    s_ix = flat_ix[order]
    s_dest = flat_dest[order]
    s_sc = flat_sc[order]

    # position of each token within its (core,bucket) group
    grp = s_core * NBUCKETS + s_bucket
    starts = np.zeros(NCORES * NBUCKETS + 1, np.int64)
    np.add.at(starts, grp + 1, 1)
    starts = np.cumsum(starts)
    within = np.arange(len(grp)) - starts[grp]
    pos = s_bucket * cap + within  # position within the core's token buffer

    gidx_all = np.full((NCORES, TOT), -1, np.int16)
    score_all = np.zeros((NCORES, TOT), np.float32)
    dest_all = np.full((NCORES, TOT), -1, np.int64)

    gidx_all[s_core, pos] = (s_ix % BUCKET).astype(np.int16)
    score_all[s_core, pos] = s_sc
    dest_all[s_core, pos] = s_dest

    # per-(core,bucket) valid counts for the gather num_idxs registers;
    # guarantee >=1 valid entry per bucket (gather of 0 idxs is invalid)
    cnt_all = counts.astype(np.int32)
    for c, k in zip(*np.nonzero(cnt_all == 0)):
        gidx_all[c, k * cap] = 0  # dummy row; score 0 / destrel -1 ignore it
        cnt_all[c, k] = 1

    # per block (128 consecutive positions): rank-compress dests
    destrel_all = np.full((NCORES, TOT), -1.0, np.float32)
    rowmaps = np.full((NCORES, G, 128), -1, np.int64)
    for c in range(NCORES):
        d = dest_all[c].reshape(G, 128)
        for g in range(G):
            blk = d[g]
            valid = blk >= 0
            if not valid.any():
                continue
            uniq, inv = np.unique(blk[valid], return_inverse=True)
            destrel_all[c, g * 128 : (g + 1) * 128][valid] = inv.astype(np.float32)
            rowmaps[c, g, : len(uniq)] = uniq

    # pair buckets onto the 4 SWDGE queues to minimize the slowest core's
    # worst queue (desc-gen ~ token count; paired calls serialize on one
    # ucode worker). Exactly 8 Pool DMAs -> each DMASW lane hosts one call,
    # so queue choice is unconstrained. Larger call goes first so the final
    # (binding) gen+transfer is the smaller one.
    # pair buckets onto the 4 SWDGE queues to balance per-queue desc-gen
    # (gen ~ token count; paired calls serialize on one ucode worker).
    # Wave-1 must stay buckets 0-3: the PE drains matmuls in layout order,
    # so early buckets must get their data first. Only the wave-2 partner
    # assignment (4! options) is optimized, over all cores' counts.
    import itertools
    best_cost, best_pi = None, None
    for pi in itertools.permutations(range(4, NBUCKETS)):
        cost = max(
            int((counts[:, a] + counts[:, pi[a]]).max()) for a in range(4)
        )
        if best_cost is None or cost < best_cost:
            best_cost, best_pi = cost, pi
    order = tuple(
        [(a, a) for a in range(4)] + [(best_pi[a], a) for a in range(4)]
    )

    weight_bf = np.ascontiguousarray(np.asarray(weight, dtype=np.float32).astype(BF16))
    iota = np.ascontiguousarray(
        np.tile(np.arange(128, dtype=np.float32), (128, 1)).astype(BF16)
    )

    in_maps = []
    for c in range(NCORES):
        sc_s = score_all[c].reshape(G, 128).T  # [128, G]
        dr_s = destrel_all[c].reshape(G, 128).T
        meta = np.ascontiguousarray(
            np.concatenate([sc_s, dr_s], axis=1).astype(np.float32)
        )
        in_maps.append(
            {
                "weight": weight_bf,
                "gidx": _wrap16(gidx_all[c]),
                "meta": meta,
                "iota128": iota,
                "cnt": np.ascontiguousarray(cnt_all[c : c + 1]),
            }
        )
    return cap, order, in_maps, rowmaps


def kernel(score, indices, dispatch, n, weight):
    global LAST_RESULTS
    from concourse import bass_utils

    assert int(np.asarray(n)) == N
    cap, order, in_maps, rowmaps = _preprocess(score, indices, dispatch, weight)

    trace = _cache.pop("_trace_next", False)
    key = (cap, order, trace)
    if key not in _cache:
        _cache[key] = _build(cap, order)
    nc = _cache[key]
    res = bass_utils.run_bass_kernel_spmd(
        nc, in_maps, core_ids=list(range(NCORES)), trace=trace
    )
    LAST_RESULTS = res

    out_full = np.zeros((B, N, D), np.float32)
    for c in range(NCORES):
        acc = np.zeros((B * NSLICE, D), np.float32)
        ot = res.results[c]["out"].astype(np.float32)  # [128, TOT]
        rm = rowmaps[c].reshape(-1)
        valid = rm >= 0
        np.add.at(acc, rm[valid], ot[:, valid].T)
        out_full[:, c * NSLICE : (c + 1) * NSLICE, :] = acc.reshape(B, NSLICE, D)
    return out_full

